# revision 35
# baseline (speedup 1.0000x reference)
"""Causal self-attention with RoPE, sharded over 8 TRN2 NeuronCores.

Sharding: core = (batch b, head-group hg). Cores 0-3 -> batch 0, cores 4-7 ->
batch 1; head-group hg = core % 4 owns heads [3*hg, 3*hg+3). Each core computes
its heads' attention and a partial output projection (w_proj column-slice);
the host sums the 4 partials per batch (the row-sharded projection's
all-reduce, done on host since full outputs are gathered anyway).

Per-core kernel, bf16 data / fp32 PSUM accumulation. Matmul outputs are
hard-capped at one PSUM bank (512 fp32/partition), so all matmuls run
512-wide; non-matmul ops (muls, adds, copies, masks, exp, divides) span
two banks per instruction to halve instruction count:
  - QKV q/k features packed in 3x128 tiles [q0|q1] [k0|k1] [q2|k2],
    processed per token-block PAIR: two 512-wide matmul groups accumulate
    into one 2-bank tile, then single 1024-wide RoPE ops run on it.
  - RoPE with signs folded into a permuted sin table: q' = (acc*cos) +
    P2T @ (acc*gsin), P2T the plain half-swap permutation, gsin =
    [+sin; -sin] per 32-half. The rotate matmuls lag one acc-group so
    the PE never waits on the DVE muls. k2 is mirrored into a 4th qkrot
    slot's low half (SBUF->SBUF DMA) for head-2's equal-base-partition
    scores.
  - V^T computed directly: vt[t,d] = sum_c xT[c,t]*wv[c,d]; two key
    tiles' results share one PSUM bank and one copy into the augmented-V
    buffer whose 65th column of ones yields the softmax denominator.
  - Attention in scores-transposed layout [keys, queries]: probsT =
    exp(K^T.T @ Q^T * 0.125) on ACT over 2-tile groups, causal
    diagonal-band masking one instruction per group (2D-affine
    gpsimd affine_select alternating with DVE multiply by precomputed
    0/1 masks). PV accumulates over key tiles, software-pipelined with
    a 2-group lookahead behind scores.
  - The divide by the softmax denominator is folded into the PSUM->SBUF
    move.
  - Projection: partial outT = wpT.T @ attnT into a persistent buffer
    (paired 2-bank tiles, 1024-wide copies), 6 large bf16 DMAs out;
    host sums the 4 partials per batch in fp32.

All SBUF tile pools open once, outside the For_i timing loop (no pool
boundaries inside an iteration); only the per-phase PSUM pools (bank
reuse: 8/8/6 banks) open inside.
"""

import numpy as np
import ml_dtypes

import concourse.bass as bass
import concourse.bacc as bacc
import concourse.tile as tile
from concourse import mybir
from concourse.bass_utils import run_bass_kernel_spmd

B, T, C, H = 2, 2048, 768, 12
D = C // H  # 64
ROPE_THETA = 10000.0
NCORES = 8
HPC = 3             # heads per core
NQF = 3             # packed q/k feature tiles: [q0|q1] [k0|k1] [q2|k2]
QB = 512            # query block (free dim of scores^T tiles)
KT = 128            # key tile (partition dim of scores^T tiles)

F32 = mybir.dt.float32
BF16 = mybir.dt.bfloat16
BF = ml_dtypes.bfloat16

# (feature-tile, half) of each head's q / k block in the packed layout
Q_POS = {0: (0, 0), 1: (0, 1), 2: (2, 0)}
K_POS = {0: (1, 0), 1: (1, 1), 2: (3, 0)}   # k2 DMA-copied to slot 3 lo-half


def _build_nc(t_len=T, loops=1, unroll=False, body=1):
    nc = bacc.Bacc("TRN2", target_bir_lowering=False, debug=False)

    xT_d = nc.dram_tensor("xT", [C, t_len], BF16, kind="ExternalInput")
    wqk_d = nc.dram_tensor("wqkT", [C, NQF * 128], BF16, kind="ExternalInput")
    wv_d = nc.dram_tensor("wvT", [C, HPC * D], BF16, kind="ExternalInput")
    wp_d = nc.dram_tensor("wpT", [HPC * D, C], BF16, kind="ExternalInput")
    cos_d = nc.dram_tensor("cosT", [128, t_len], F32, kind="ExternalInput")
    gsin_d = nc.dram_tensor("gsinT", [128, t_len], F32, kind="ExternalInput")
    p2t_d = nc.dram_tensor("p2t", [128, 128], BF16, kind="ExternalInput")
    msk_d = nc.dram_tensor("msk", [128, 4 * QB], BF16, kind="ExternalInput")
    outT_d = nc.dram_tensor("outT", [C, t_len], BF16, kind="ExternalOutput")

    with tile.TileContext(nc) as tc:
        _body(tc, t_len, xT_d, wqk_d, wv_d, wp_d, cos_d, gsin_d, p2t_d, msk_d,
              outT_d, loops=loops, unroll=unroll, body=body)
    nc.compile()
    return nc


def _body(tc, t_len, xT_d, wqk_d, wv_d, wp_d, cos_d, gsin_d, p2t_d, msk_d,
          outT_d, loops=1, unroll=False, body=1):
    T = t_len
    NCT = C // 128
    NKT = T // KT
    with (
        tc.tile_pool(name="singles", bufs=1) as singles,
        tc.tile_pool(name="sb_x", bufs=2) as sb_x,
        tc.tile_pool(name="sb_qs", bufs=3) as sb_qs,
        tc.tile_pool(name="sb_qc", bufs=3) as sb_qc,
        tc.tile_pool(name="sb_probs", bufs=6) as sb_probs,
        tc.tile_pool(name="sb_rcp", bufs=2) as sb_rcp,
    ):
        s = {}
        s["wqk"] = singles.tile([128, NCT, NQF * 128], BF16, tag="wqk", name="wqk")
        s["wv"] = singles.tile([128, NCT, HPC * D], BF16, tag="wv", name="wv")
        s["wp0"] = singles.tile([128, C], BF16, tag="wp0", name="wp0")
        s["wp1"] = singles.tile([64, C], BF16, tag="wp1", name="wp1")
        s["cosc"] = singles.tile([128, T], F32, tag="cosc", name="cosc")
        s["gsin"] = singles.tile([128, T], F32, tag="gsin", name="gsin")
        s["p2t"] = singles.tile([128, 128], BF16, tag="p2t", name="p2t")
        s["msk"] = singles.tile([128, 4, QB], BF16, tag="msk", name="msk")
        s["qkrot"] = singles.tile([128, NQF + 1, T], BF16, tag="qkrot", name="qkrot")
        s["va"] = singles.tile([128, NKT, HPC, D + 1], BF16, tag="va", name="va")
        s["at01"] = singles.tile([128, T], BF16, tag="at01", name="at01")
        s["at2"] = singles.tile([64, T], BF16, tag="at2", name="at2")
        s["outb"] = singles.tile([128, C // 128, T], BF16, tag="outb", name="outb")
        pools = dict(sb_x=sb_x, sb_qs=sb_qs, sb_qc=sb_qc, sb_probs=sb_probs,
                     sb_rcp=sb_rcp)

        # pre-zero the probs ring once so trimmed-range exp never leaves
        # non-finite garbage in masked regions (mask-multiply safe)
        for i in range(6):
            pz = sb_probs.tile([128, 2, QB], BF16, tag="probs", name=f"pz{i}")
            nc = tc.nc
            nc.vector.memset(pz, 0.0)

        if loops > 1 and unroll:
            for _ in range(loops * body):
                _compute(tc, t_len, s, pools, xT_d, wqk_d, wv_d, wp_d, cos_d,
                         gsin_d, p2t_d, msk_d, outT_d)
        elif loops > 1:
            with tc.For_i(0, loops, 1):
                for _ in range(body):
                    _compute(tc, t_len, s, pools, xT_d, wqk_d, wv_d, wp_d,
                             cos_d, gsin_d, p2t_d, msk_d, outT_d)
        else:
            for _ in range(body):
                _compute(tc, t_len, s, pools, xT_d, wqk_d, wv_d, wp_d, cos_d,
                         gsin_d, p2t_d, msk_d, outT_d)


def _compute(tc, t_len, s, pools, xT_d, wqk_d, wv_d, wp_d, cos_d, gsin_d,
             p2t_d, msk_d, outT_d):
    nc = tc.nc
    T = t_len
    NQB = T // QB       # 4 query blocks
    NKT = T // KT       # 16 key tiles
    NCT = C // 128      # 6 contraction tiles over channels
    JPB = QB // KT      # key tiles per query block (4)
    QB2 = 2 * QB        # elementwise ops span two 512 tiles
    NTB2 = T // QB2     # 2 token-block pairs

    wqk, wv, wp0, wp1 = s["wqk"], s["wv"], s["wp0"], s["wp1"]
    cosc, gsin, p2t, msk = s["cosc"], s["gsin"], s["p2t"], s["msk"]
    qkrot, va, at01, at2, outb = (s["qkrot"], s["va"], s["at01"], s["at2"],
                                  s["outb"])
    sb_x, sb_qs, sb_qc = pools["sb_x"], pools["sb_qs"], pools["sb_qc"]
    sb_probs, sb_rcp = pools["sb_probs"], pools["sb_rcp"]

    wqk_v = wqk_d.ap().rearrange("(a p) f -> p a f", p=128)
    wv_v = wv_d.ap().rearrange("(a p) f -> p a f", p=128)
    xT_v = xT_d.ap().rearrange("(a p) t -> p a t", p=128)

    xtbs = []
    for tb in range(NQB):
        xtb = sb_x.tile([128, NCT, QB], BF16, tag="xtb", name=f"xtb{tb}")
        xtbs.append(xtb)
    # critical-path order: weights + x block 0 + rope tables first, the
    # remaining x blocks next, cold constants trailing
    nc.sync.dma_start(out=wqk, in_=wqk_v)
    nc.sync.dma_start(out=xtbs[0], in_=xT_v[:, :, 0:QB])
    nc.sync.dma_start(out=cosc, in_=cos_d.ap())
    nc.sync.dma_start(out=gsin, in_=gsin_d.ap())
    nc.sync.dma_start(out=p2t, in_=p2t_d.ap())
    nc.sync.dma_start(out=wv, in_=wv_v)
    nc.sync.dma_start(out=xtbs[1], in_=xT_v[:, :, QB:2 * QB])
    nc.sync.dma_start(out=xtbs[2], in_=xT_v[:, :, 2 * QB:3 * QB])
    nc.sync.dma_start(out=xtbs[3], in_=xT_v[:, :, 3 * QB:4 * QB])
    nc.sync.dma_start(out=msk, in_=msk_d.ap().rearrange("p (a q) -> p a q", q=QB))
    nc.sync.dma_start(out=wp0, in_=wp_d.ap()[0:128, :])
    nc.sync.dma_start(out=wp1, in_=wp_d.ap()[128:192, :])

    # ones column of the augmented V tiles (softmax denominator)
    nc.vector.memset(va[:, :, :, D], 1.0)

    # ---- QKV projection + RoPE + direct V^T ------------------------------
    with (
        tc.tile_pool(name="ps_a", bufs=2, space="PSUM") as ps_a,
        tc.tile_pool(name="ps_r", bufs=2, space="PSUM") as ps_r,
        tc.tile_pool(name="ps_v", bufs=3, space="PSUM") as ps_v,
    ):
        # software pipeline: the rotate matmul for (tb, ft) is emitted one
        # acc-group later so the PE never waits on the DVE mul feeding it
        prev = []   # at most one (qs, qc, ft, tb) awaiting rotate

        def flush_rot():
            qs, qc, ft, tb = prev.pop(0)
            ts = slice(tb * QB, (tb + 1) * QB)
            rh = ps_r.tile([128, QB], F32, tag="rh", name="rh")
            nc.tensor.matmul(rh, p2t, qs, start=True, stop=True)
            nc.vector.tensor_add(qkrot[:, ft, ts], qc, rh)
            if ft == 2:
                # scores need k2 on the same base partition as q2:
                # mirror the hi half into slot 3's lo half
                nc.sync.dma_start(out=qkrot[0:64, 3, ts],
                                  in_=qkrot[64:128, 2, ts])

        for tb in range(NQB):
            ts = slice(tb * QB, (tb + 1) * QB)
            xtb = xtbs[tb]
            for ft in range(NQF):
                acc = ps_a.tile([128, QB], F32, tag="acc", name="acc")
                for ct in range(NCT):
                    nc.tensor.matmul(
                        acc,
                        wqk[:, ct, ft * 128: (ft + 1) * 128],
                        xtb[:, ct, :],
                        start=(ct == 0),
                        stop=(ct == NCT - 1),
                    )
                if prev:
                    flush_rot()
                qs = sb_qs.tile([128, QB], BF16, tag="qs", name="qs")
                nc.vector.tensor_mul(qs, acc, gsin[:, ts])
                qc = sb_qc.tile([128, QB], BF16, tag="qc", name="qc")
                nc.vector.tensor_mul(qc, acc, cosc[:, ts])
                prev.append((qs, qc, ft, tb))
            # direct V^T for this token block (4 x 128-token tiles, two
            # per PSUM bank / per copy)
            for j2 in range(JPB // 2):
                kt = tb * JPB + 2 * j2
                vt = ps_v.tile([128, 2, HPC * D], F32, tag="vt", name="vt")
                for u in range(2):
                    for ct in range(NCT):
                        nc.tensor.matmul(
                            vt[:, u, :],
                            xtb[:, ct, (2 * j2 + u) * KT: (2 * j2 + u + 1) * KT],
                            wv[:, ct, :],
                            start=(ct == 0),
                            stop=(ct == NCT - 1),
                        )
                if j2 == 0 and prev:
                    flush_rot()
                nc.scalar.copy(
                    va[:, kt: kt + 2, :, 0:D],
                    vt.rearrange("p a (h d) -> p a h d", h=HPC),
                )
        while prev:
            flush_rot()

    # ---- attention -------------------------------------------------------
    def qk_ap(pos, ts):
        ti, half = pos
        return qkrot[half * 64: half * 64 + 64, ti, ts]

    with (
        tc.tile_pool(name="ps_sc", bufs=3, space="PSUM") as ps_sc,
        tc.tile_pool(name="ps_pv", bufs=2, space="PSUM") as ps_pv,
    ):
        pvs = {}     # (h, qb) -> pv tile
        pend = []    # (h, qb, g, probs2, nkt) score groups awaiting PV
        nmask = [0]  # running diag-mask count for engine alternation

        def emit_pv(depth):
            while len(pend) > depth:
                h, qb, g, probs2, nkt = pend.pop(0)
                pv = pvs[(h, qb)]
                for j2 in range(2):
                    kt = g * 2 + j2
                    nc.tensor.matmul(
                        pv,
                        va[:, kt, h, :],
                        probs2[:, j2, :],
                        start=(kt == 0),
                        stop=(kt == nkt - 1),
                    )
                if g == nkt // 2 - 1:
                    # block done: fold denominator into the PSUM->SBUF move
                    rcp = sb_rcp.tile([1, QB], F32, tag="rcp", name="rcp")
                    nc.vector.reciprocal(rcp, pv[64:65, :])
                    rcpb = sb_rcp.tile([64, QB], F32, tag="rcpb", name="rcpb")
                    nc.gpsimd.partition_broadcast(rcpb, rcp)
                    qs_ = slice(qb * QB, (qb + 1) * QB)
                    if h == 0:
                        dst = at01[0:64, qs_]
                    elif h == 1:
                        dst = at01[64:128, qs_]
                    else:
                        dst = at2[:, qs_]
                    nc.vector.tensor_mul(dst, pv[0:64, :], rcpb)
                    del pvs[(h, qb)]

        for h in range(HPC):
            for qb in range(NQB):
                qs_ = slice(qb * QB, (qb + 1) * QB)
                nkt = (qb + 1) * JPB
                pvs[(h, qb)] = ps_pv.tile([65, QB], F32, tag="pv",
                                          name=f"pv{h}_{qb}")
                for g in range(nkt // 2):
                    sc2 = ps_sc.tile([128, 2, QB], F32, tag="sc2", name="sc2")
                    for j2 in range(2):
                        kt = g * 2 + j2
                        nc.tensor.matmul(
                            sc2[:, j2, :],
                            qk_ap(K_POS[h], slice(kt * KT, (kt + 1) * KT)),
                            qk_ap(Q_POS[h], qs_),
                            start=True, stop=True,
                        )
                    probs2 = sb_probs.tile([128, 2, QB], BF16, tag="probs",
                                           name="probs")
                    p0 = g * 2 - qb * JPB  # diag-band pattern of tile j2=0
                    # diag-band groups: columns < 128*p0 are masked in BOTH
                    # tiles, so exp only [128*p0, 512) of each; the
                    # affine_select zero-fill covers the unwritten range
                    lo = max(p0, 0) * KT
                    nc.scalar.activation(
                        probs2[:, :, lo:QB], sc2[:, :, lo:QB],
                        mybir.ActivationFunctionType.Exp,
                        scale=float(1.0 / np.sqrt(D)),
                    )
                    for j2 in range(2):
                        kt = g * 2 + j2
                        p = kt - qb * JPB  # diag-band pattern id
                        if p >= 0:
                            if nmask[0] % 3 != 0:
                                nc.gpsimd.affine_select(
                                    out=probs2[:, j2, :],
                                    in_=probs2[:, j2, :],
                                    compare_op=mybir.AluOpType.is_ge,
                                    fill=0.0, base=-p * KT,
                                    pattern=[[1, QB]],
                                    channel_multiplier=-1,
                                )
                            else:
                                nc.vector.tensor_mul(
                                    probs2[:, j2, :],
                                    probs2[:, j2, :],
                                    msk[:, p, :],
                                )
                            nmask[0] += 1
                    pend.append((h, qb, g, probs2, nkt))
                    emit_pv(2)
        emit_pv(0)

    # ---- output projection (partial over this core's 192 channels) -------
    with tc.tile_pool(name="ps_po", bufs=4, space="PSUM") as ps_po:
        k = 0
        for co in range(C // 128):
            for tb in range(NQB):
                ts = slice(tb * QB, (tb + 1) * QB)
                po = ps_po.tile([128, QB], F32, tag="po", name="po")
                nc.tensor.matmul(
                    po, wp0[:, co * 128: (co + 1) * 128],
                    at01[:, ts], start=True, stop=False,
                )
                nc.tensor.matmul(
                    po, wp1[:, co * 128: (co + 1) * 128],
                    at2[:, ts], start=False, stop=True,
                )
                ot = outb[:, co, ts]
                if k % 2 == 0:
                    nc.vector.tensor_copy(ot, po)
                else:
                    nc.scalar.copy(ot, po)
                k += 1
            nc.scalar.dma_start(
                out=outT_d.ap()[co * 128: (co + 1) * 128, :],
                in_=outb[:, co, :],
            )


_NC_CACHE = {}


def _get_nc():
    if "nc" not in _NC_CACHE:
        _NC_CACHE["nc"] = _build_nc()
    return _NC_CACHE["nc"]


def _host_consts(t_len=T):
    half = D // 2  # 32
    inv_freq = 1.0 / (ROPE_THETA ** (np.arange(0, D, 2, dtype=np.float32) / D))
    ang = np.arange(t_len, dtype=np.float32)[:, None] * inv_freq[None, :]
    sin = np.sin(ang).T.astype(np.float32)   # (32, T)
    cos = np.cos(ang).T.astype(np.float32)   # (32, T)
    cos64 = np.concatenate([cos, cos], axis=0)            # (64, T)
    gsin64 = np.concatenate([sin, -sin], axis=0)          # (64, T)
    cos128 = np.concatenate([cos64, cos64], axis=0)       # (128, T)
    gsin128 = np.concatenate([gsin64, gsin64], axis=0)    # (128, T)
    # plain half-swap permutation per 64-feature block:
    # out[m] = in[sigma(m)], sigma swaps 32-halves; p2t[sigma(m), m] = 1
    P64 = np.zeros((D, D), dtype=np.float32)
    P64[np.arange(half), np.arange(half) + half] = 1.0
    P64[np.arange(half) + half, np.arange(half)] = 1.0
    p2t = np.zeros((128, 128), dtype=np.float32)
    p2t[0:D, 0:D] = P64
    p2t[D:128, D:128] = P64
    # diag-band masks: msk[p][key, q] = 1 iff q - key >= 128*p
    k_idx = np.arange(KT)[:, None]
    q_idx = np.arange(QB)[None, :]
    msk = np.stack(
        [(q_idx - k_idx >= 128 * p).astype(np.float32) for p in range(4)],
        axis=1,
    ).reshape(KT, 4 * QB)
    return cos128, gsin128, p2t.astype(BF), msk.astype(BF)


def _pack_w(w_qkv, heads):
    """Pack this core's q/k rows into the (384, C) tile layout and v rows
    into (192, C)."""
    q = [w_qkv[0 * C + h * D: 0 * C + (h + 1) * D] for h in heads]
    kk = [w_qkv[1 * C + h * D: 1 * C + (h + 1) * D] for h in heads]
    v = [w_qkv[2 * C + h * D: 2 * C + (h + 1) * D] for h in heads]
    wqk = np.concatenate([q[0], q[1], kk[0], kk[1], q[2], kk[2]], axis=0)
    wv = np.concatenate(v, axis=0)
    return wqk, wv


def _make_in_maps(x, w_qkv, w_proj, t_len=T):
    cos128, gsin128, p2t, msk = _host_consts(t_len)
    in_maps = []
    for core in range(NCORES):
        b, hg = divmod(core, 4)
        heads = list(range(hg * HPC, (hg + 1) * HPC))
        wqk, wv = _pack_w(w_qkv, heads)
        cs = slice(hg * HPC * D, (hg + 1) * HPC * D)
        in_maps.append(
            {
                "xT": np.ascontiguousarray(x[b].T).astype(BF),
                "wqkT": np.ascontiguousarray(wqk.T).astype(BF),
                "wvT": np.ascontiguousarray(wv.T).astype(BF),
                "wpT": np.ascontiguousarray(w_proj[:, cs].T).astype(BF),
                "cosT": cos128, "gsinT": gsin128, "p2t": p2t, "msk": msk,
            }
        )
    return in_maps


def kernel(x, w_qkv, w_proj):
    x = np.asarray(x, dtype=np.float32)
    w_qkv = np.asarray(w_qkv, dtype=np.float32)
    w_proj = np.asarray(w_proj, dtype=np.float32)

    in_maps = _make_in_maps(x, w_qkv, w_proj)
    nc = _get_nc()
    res = run_bass_kernel_spmd(nc, in_maps, core_ids=list(range(NCORES)))
    out = np.zeros((B, T, C), dtype=np.float32)
    for core in range(NCORES):
        b = core // 4
        out[b] += res.results[core]["outT"].T.astype(np.float32)
    return out


# revision 36
# speedup vs baseline: 1.0415x; 1.0415x over previous
"""Causal self-attention with RoPE, sharded over 8 TRN2 NeuronCores.

Sharding: core = (batch b, head-group hg). Cores 0-3 -> batch 0, cores 4-7 ->
batch 1; head-group hg = core % 4 owns heads [3*hg, 3*hg+3). Each core computes
its heads' attention and a partial output projection (w_proj column-slice);
the host sums the 4 partials per batch (the row-sharded projection's
all-reduce, done on host since full outputs are gathered anyway).

Per-core kernel, bf16 data / fp32 PSUM accumulation. Matmul outputs are
hard-capped at one PSUM bank (512 fp32/partition), so all matmuls run
512-wide; non-matmul ops (muls, adds, copies, masks, exp, divides) span
two banks per instruction to halve instruction count:
  - QKV q/k features packed in 3x128 tiles [q0|q1] [k0|k1] [q2|k2],
    processed per token-block PAIR: two 512-wide matmul groups accumulate
    into one 2-bank tile, then single 1024-wide RoPE ops run on it.
  - RoPE with signs folded into a permuted sin table: q' = (acc*cos) +
    P2T @ (acc*gsin), P2T the plain half-swap permutation, gsin =
    [+sin; -sin] per 32-half. The rotate matmuls lag one acc-group so
    the PE never waits on the DVE muls. k2 is mirrored into a 4th qkrot
    slot's low half (SBUF->SBUF DMA) for head-2's equal-base-partition
    scores.
  - V^T computed directly: vt[t,d] = sum_c xT[c,t]*wv[c,d]; two key
    tiles' results share one PSUM bank and one copy into the augmented-V
    buffer whose 65th column of ones yields the softmax denominator.
  - Attention in scores-transposed layout [keys, queries]: probsT =
    exp(K^T.T @ Q^T * 0.125) on ACT over 2-tile groups, causal
    diagonal-band masking one instruction per group (2D-affine
    gpsimd affine_select alternating with DVE multiply by precomputed
    0/1 masks). PV accumulates over key tiles, software-pipelined with
    a 2-group lookahead behind scores.
  - The divide by the softmax denominator is folded into the PSUM->SBUF
    move.
  - Projection: partial outT = wpT.T @ attnT into a persistent buffer
    (paired 2-bank tiles, 1024-wide copies), 6 large bf16 DMAs out;
    host sums the 4 partials per batch in fp32.

All SBUF tile pools open once, outside the For_i timing loop (no pool
boundaries inside an iteration); only the per-phase PSUM pools (bank
reuse: 8/8/6 banks) open inside.
"""

import numpy as np
import ml_dtypes

import concourse.bass as bass
import concourse.bacc as bacc
import concourse.tile as tile
from concourse import mybir
from concourse.bass_utils import run_bass_kernel_spmd

B, T, C, H = 2, 2048, 768, 12
D = C // H  # 64
ROPE_THETA = 10000.0
NCORES = 8
HPC = 3             # heads per core
NQF = 3             # packed q/k feature tiles: [q0|q1] [k0|k1] [q2|k2]
QB = 512            # query block (free dim of scores^T tiles)
KT = 128            # key tile (partition dim of scores^T tiles)

F32 = mybir.dt.float32
BF16 = mybir.dt.bfloat16
BF = ml_dtypes.bfloat16

# (feature-tile, half) of each head's q / k block in the packed layout
Q_POS = {0: (0, 0), 1: (0, 1), 2: (2, 0)}
K_POS = {0: (1, 0), 1: (1, 1), 2: (3, 0)}   # k2 DMA-copied to slot 3 lo-half


def _build_nc(t_len=T, loops=1, unroll=False, body=1):
    nc = bacc.Bacc("TRN2", target_bir_lowering=False, debug=False)

    xT_d = nc.dram_tensor("xT", [C, t_len], BF16, kind="ExternalInput")
    wqk_d = nc.dram_tensor("wqkT", [C, NQF * 128], BF16, kind="ExternalInput")
    wv_d = nc.dram_tensor("wvT", [C, HPC * D], BF16, kind="ExternalInput")
    wp_d = nc.dram_tensor("wpT", [HPC * D, C], BF16, kind="ExternalInput")
    cos_d = nc.dram_tensor("cosT", [128, t_len], F32, kind="ExternalInput")
    gsin_d = nc.dram_tensor("gsinT", [128, t_len], F32, kind="ExternalInput")
    p2t_d = nc.dram_tensor("p2t", [128, 128], BF16, kind="ExternalInput")
    msk_d = nc.dram_tensor("msk", [128, 4 * QB], BF16, kind="ExternalInput")
    outT_d = nc.dram_tensor("outT", [C, t_len], BF16, kind="ExternalOutput")

    with tile.TileContext(nc) as tc:
        _body(tc, t_len, xT_d, wqk_d, wv_d, wp_d, cos_d, gsin_d, p2t_d, msk_d,
              outT_d, loops=loops, unroll=unroll, body=body)
    nc.compile()
    return nc


def _body(tc, t_len, xT_d, wqk_d, wv_d, wp_d, cos_d, gsin_d, p2t_d, msk_d,
          outT_d, loops=1, unroll=False, body=1):
    T = t_len
    NCT = C // 128
    NKT = T // KT
    with (
        tc.tile_pool(name="singles", bufs=1) as singles,
        tc.tile_pool(name="sb_x", bufs=2) as sb_x,
        tc.tile_pool(name="sb_qs", bufs=3) as sb_qs,
        tc.tile_pool(name="sb_qc", bufs=3) as sb_qc,
        tc.tile_pool(name="sb_probs", bufs=6) as sb_probs,
        tc.tile_pool(name="sb_rcp", bufs=2) as sb_rcp,
    ):
        s = {}
        s["wqk"] = singles.tile([128, NCT, NQF * 128], BF16, tag="wqk", name="wqk")
        s["wv"] = singles.tile([128, NCT, HPC * D], BF16, tag="wv", name="wv")
        s["wp0"] = singles.tile([128, C], BF16, tag="wp0", name="wp0")
        s["wp1"] = singles.tile([64, C], BF16, tag="wp1", name="wp1")
        s["cosc"] = singles.tile([128, T], F32, tag="cosc", name="cosc")
        s["gsin"] = singles.tile([128, T], F32, tag="gsin", name="gsin")
        s["p2t"] = singles.tile([128, 128], BF16, tag="p2t", name="p2t")
        s["msk"] = singles.tile([128, 4, QB], BF16, tag="msk", name="msk")
        s["qkrot"] = singles.tile([128, NQF + 1, T], BF16, tag="qkrot", name="qkrot")
        s["va"] = singles.tile([128, NKT, HPC, D + 1], BF16, tag="va", name="va")
        s["at01"] = singles.tile([128, T], BF16, tag="at01", name="at01")
        s["at2"] = singles.tile([64, T], BF16, tag="at2", name="at2")
        s["outb"] = singles.tile([128, C // 128, T], BF16, tag="outb", name="outb")
        pools = dict(sb_x=sb_x, sb_qs=sb_qs, sb_qc=sb_qc, sb_probs=sb_probs,
                     sb_rcp=sb_rcp)

        # pre-zero the probs ring once so trimmed-range exp never leaves
        # non-finite garbage in masked regions (mask-multiply safe)
        for i in range(6):
            pz = sb_probs.tile([128, 2, QB], BF16, tag="probs", name=f"pz{i}")
            nc = tc.nc
            nc.vector.memset(pz, 0.0)

        if loops > 1 and unroll:
            for _ in range(loops * body):
                _compute(tc, t_len, s, pools, xT_d, wqk_d, wv_d, wp_d, cos_d,
                         gsin_d, p2t_d, msk_d, outT_d)
        elif loops > 1:
            with tc.For_i(0, loops, 1):
                for _ in range(body):
                    _compute(tc, t_len, s, pools, xT_d, wqk_d, wv_d, wp_d,
                             cos_d, gsin_d, p2t_d, msk_d, outT_d)
        else:
            for _ in range(body):
                _compute(tc, t_len, s, pools, xT_d, wqk_d, wv_d, wp_d, cos_d,
                         gsin_d, p2t_d, msk_d, outT_d)


def _compute(tc, t_len, s, pools, xT_d, wqk_d, wv_d, wp_d, cos_d, gsin_d,
             p2t_d, msk_d, outT_d):
    nc = tc.nc
    T = t_len
    NQB = T // QB       # 4 query blocks
    NKT = T // KT       # 16 key tiles
    NCT = C // 128      # 6 contraction tiles over channels
    JPB = QB // KT      # key tiles per query block (4)
    QB2 = 2 * QB        # elementwise ops span two 512 tiles
    NTB2 = T // QB2     # 2 token-block pairs

    wqk, wv, wp0, wp1 = s["wqk"], s["wv"], s["wp0"], s["wp1"]
    cosc, gsin, p2t, msk = s["cosc"], s["gsin"], s["p2t"], s["msk"]
    qkrot, va, at01, at2, outb = (s["qkrot"], s["va"], s["at01"], s["at2"],
                                  s["outb"])
    sb_x, sb_qs, sb_qc = pools["sb_x"], pools["sb_qs"], pools["sb_qc"]
    sb_probs, sb_rcp = pools["sb_probs"], pools["sb_rcp"]

    wqk_v = wqk_d.ap().rearrange("(a p) f -> p a f", p=128)
    wv_v = wv_d.ap().rearrange("(a p) f -> p a f", p=128)
    xT_v = xT_d.ap().rearrange("(a p) t -> p a t", p=128)

    xtbs = []
    for tb in range(NQB):
        xtb = sb_x.tile([128, NCT, QB], BF16, tag="xtb", name=f"xtb{tb}")
        xtbs.append(xtb)
    # critical-path order: weights + x block 0 + rope tables first, the
    # remaining x blocks next, cold constants trailing
    nc.sync.dma_start(out=wqk, in_=wqk_v)
    nc.sync.dma_start(out=xtbs[0], in_=xT_v[:, :, 0:QB])
    nc.sync.dma_start(out=cosc, in_=cos_d.ap())
    nc.sync.dma_start(out=gsin, in_=gsin_d.ap())
    nc.sync.dma_start(out=p2t, in_=p2t_d.ap())
    nc.sync.dma_start(out=wv, in_=wv_v)
    nc.sync.dma_start(out=xtbs[1], in_=xT_v[:, :, QB:2 * QB])
    nc.sync.dma_start(out=xtbs[2], in_=xT_v[:, :, 2 * QB:3 * QB])
    nc.sync.dma_start(out=xtbs[3], in_=xT_v[:, :, 3 * QB:4 * QB])
    nc.sync.dma_start(out=msk, in_=msk_d.ap().rearrange("p (a q) -> p a q", q=QB))
    nc.sync.dma_start(out=wp0, in_=wp_d.ap()[0:128, :])
    nc.sync.dma_start(out=wp1, in_=wp_d.ap()[128:192, :])

    # ones column of the augmented V tiles (softmax denominator)
    nc.vector.memset(va[:, :, :, D], 1.0)

    # ---- QKV projection + RoPE + direct V^T ------------------------------
    with (
        tc.tile_pool(name="ps_a", bufs=2, space="PSUM") as ps_a,
        tc.tile_pool(name="ps_r", bufs=2, space="PSUM") as ps_r,
        tc.tile_pool(name="ps_v", bufs=3, space="PSUM") as ps_v,
    ):
        # software pipeline: the rotate matmul for (tb, ft) is emitted one
        # acc-group later so the PE never waits on the DVE mul feeding it
        prev = []   # at most one (qs, qc, ft, tb) awaiting rotate

        def flush_rot():
            qs, qc, ft, tb = prev.pop(0)
            ts = slice(tb * QB, (tb + 1) * QB)
            rh = ps_r.tile([128, QB], F32, tag="rh", name="rh")
            nc.tensor.matmul(rh, p2t, qs, start=True, stop=True)
            nc.vector.tensor_add(qkrot[:, ft, ts], qc, rh)
            if ft == 2:
                # scores need k2 on the same base partition as q2:
                # mirror the hi half into slot 3's lo half
                nc.sync.dma_start(out=qkrot[0:64, 3, ts],
                                  in_=qkrot[64:128, 2, ts])

        for tb in range(NQB):
            ts = slice(tb * QB, (tb + 1) * QB)
            xtb = xtbs[tb]
            for ft in range(NQF):
                acc = ps_a.tile([128, QB], F32, tag="acc", name="acc")
                for ct in range(NCT):
                    nc.tensor.matmul(
                        acc,
                        wqk[:, ct, ft * 128: (ft + 1) * 128],
                        xtb[:, ct, :],
                        start=(ct == 0),
                        stop=(ct == NCT - 1),
                    )
                if prev:
                    flush_rot()
                qs = sb_qs.tile([128, QB], BF16, tag="qs", name="qs")
                nc.vector.tensor_mul(qs, acc, gsin[:, ts])
                qc = sb_qc.tile([128, QB], BF16, tag="qc", name="qc")
                nc.vector.tensor_mul(qc, acc, cosc[:, ts])
                prev.append((qs, qc, ft, tb))
            # direct V^T for this token block (4 x 128-token tiles, two
            # per PSUM bank / per copy)
            for j2 in range(JPB // 2):
                kt = tb * JPB + 2 * j2
                vt = ps_v.tile([128, 2, HPC * D], F32, tag="vt", name="vt")
                for u in range(2):
                    for ct in range(NCT):
                        nc.tensor.matmul(
                            vt[:, u, :],
                            xtb[:, ct, (2 * j2 + u) * KT: (2 * j2 + u + 1) * KT],
                            wv[:, ct, :],
                            start=(ct == 0),
                            stop=(ct == NCT - 1),
                        )
                if j2 == 0 and prev:
                    flush_rot()
                nc.scalar.copy(
                    va[:, kt: kt + 2, :, 0:D],
                    vt.rearrange("p a (h d) -> p a h d", h=HPC),
                )
        while prev:
            flush_rot()

    # ---- attention -------------------------------------------------------
    def qk_ap(pos, ts):
        ti, half = pos
        return qkrot[half * 64: half * 64 + 64, ti, ts]

    with (
        tc.tile_pool(name="ps_sc", bufs=3, space="PSUM") as ps_sc,
        tc.tile_pool(name="ps_pv", bufs=2, space="PSUM") as ps_pv,
    ):
        pvs = {}     # (h, qb) -> pv tile
        pend = []    # (h, qb, g, probs2, nkt) score groups awaiting PV
        nmask = [0]  # running diag-mask count for engine alternation

        def emit_pv(depth):
            while len(pend) > depth:
                h, qb, g, probs2, nkt = pend.pop(0)
                pv = pvs[(h, qb)]
                for j2 in range(2):
                    kt = g * 2 + j2
                    nc.tensor.matmul(
                        pv,
                        va[:, kt, h, :],
                        probs2[:, j2, :],
                        start=(kt == 0),
                        stop=(kt == nkt - 1),
                    )
                if g == nkt // 2 - 1:
                    # block done: fold denominator into the PSUM->SBUF move
                    rcp = sb_rcp.tile([1, QB], F32, tag="rcp", name="rcp")
                    nc.vector.reciprocal(rcp, pv[64:65, :])
                    rcpb = sb_rcp.tile([64, QB], F32, tag="rcpb", name="rcpb")
                    nc.gpsimd.partition_broadcast(rcpb, rcp)
                    qs_ = slice(qb * QB, (qb + 1) * QB)
                    if h == 0:
                        dst = at01[0:64, qs_]
                    elif h == 1:
                        dst = at01[64:128, qs_]
                    else:
                        dst = at2[:, qs_]
                    nc.vector.tensor_mul(dst, pv[0:64, :], rcpb)
                    del pvs[(h, qb)]

        for h in range(HPC):
            for qb in range(NQB):
                qs_ = slice(qb * QB, (qb + 1) * QB)
                nkt = (qb + 1) * JPB
                pvs[(h, qb)] = ps_pv.tile([65, QB], F32, tag="pv",
                                          name=f"pv{h}_{qb}")
                for g in range(nkt // 2):
                    sc2 = ps_sc.tile([128, 2, QB], F32, tag="sc2", name="sc2")
                    for j2 in range(2):
                        kt = g * 2 + j2
                        nc.tensor.matmul(
                            sc2[:, j2, :],
                            qk_ap(K_POS[h], slice(kt * KT, (kt + 1) * KT)),
                            qk_ap(Q_POS[h], qs_),
                            start=True, stop=True,
                        )
                    probs2 = sb_probs.tile([128, 2, QB], BF16, tag="probs",
                                           name="probs")
                    p0 = g * 2 - qb * JPB  # diag-band pattern of tile j2=0
                    # diag-band groups: columns < 128*p0 are masked in BOTH
                    # tiles, so exp only [128*p0, 512) of each; the
                    # affine_select zero-fill covers the unwritten range
                    lo = max(p0, 0) * KT
                    nc.scalar.activation(
                        probs2[:, :, lo:QB], sc2[:, :, lo:QB],
                        mybir.ActivationFunctionType.Exp,
                        scale=float(1.0 / np.sqrt(D)),
                    )
                    for j2 in range(2):
                        kt = g * 2 + j2
                        p = kt - qb * JPB  # diag-band pattern id
                        if p >= 0:
                            if nmask[0] % 4 == 0:
                                nc.gpsimd.affine_select(
                                    out=probs2[:, j2, :],
                                    in_=probs2[:, j2, :],
                                    compare_op=mybir.AluOpType.is_ge,
                                    fill=0.0, base=-p * KT,
                                    pattern=[[1, QB]],
                                    channel_multiplier=-1,
                                )
                            else:
                                nc.vector.tensor_mul(
                                    probs2[:, j2, :],
                                    probs2[:, j2, :],
                                    msk[:, p, :],
                                )
                            nmask[0] += 1
                    pend.append((h, qb, g, probs2, nkt))
                    emit_pv(2)
        emit_pv(0)

    # ---- output projection (partial over this core's 192 channels) -------
    with tc.tile_pool(name="ps_po", bufs=4, space="PSUM") as ps_po:
        k = 0
        for co in range(C // 128):
            for tb in range(NQB):
                ts = slice(tb * QB, (tb + 1) * QB)
                po = ps_po.tile([128, QB], F32, tag="po", name="po")
                nc.tensor.matmul(
                    po, wp0[:, co * 128: (co + 1) * 128],
                    at01[:, ts], start=True, stop=False,
                )
                nc.tensor.matmul(
                    po, wp1[:, co * 128: (co + 1) * 128],
                    at2[:, ts], start=False, stop=True,
                )
                ot = outb[:, co, ts]
                if k % 2 == 0:
                    nc.vector.tensor_copy(ot, po)
                else:
                    nc.scalar.copy(ot, po)
                k += 1
            nc.scalar.dma_start(
                out=outT_d.ap()[co * 128: (co + 1) * 128, :],
                in_=outb[:, co, :],
            )


_NC_CACHE = {}


def _get_nc():
    if "nc" not in _NC_CACHE:
        _NC_CACHE["nc"] = _build_nc()
    return _NC_CACHE["nc"]


def _host_consts(t_len=T):
    half = D // 2  # 32
    inv_freq = 1.0 / (ROPE_THETA ** (np.arange(0, D, 2, dtype=np.float32) / D))
    ang = np.arange(t_len, dtype=np.float32)[:, None] * inv_freq[None, :]
    sin = np.sin(ang).T.astype(np.float32)   # (32, T)
    cos = np.cos(ang).T.astype(np.float32)   # (32, T)
    cos64 = np.concatenate([cos, cos], axis=0)            # (64, T)
    gsin64 = np.concatenate([sin, -sin], axis=0)          # (64, T)
    cos128 = np.concatenate([cos64, cos64], axis=0)       # (128, T)
    gsin128 = np.concatenate([gsin64, gsin64], axis=0)    # (128, T)
    # plain half-swap permutation per 64-feature block:
    # out[m] = in[sigma(m)], sigma swaps 32-halves; p2t[sigma(m), m] = 1
    P64 = np.zeros((D, D), dtype=np.float32)
    P64[np.arange(half), np.arange(half) + half] = 1.0
    P64[np.arange(half) + half, np.arange(half)] = 1.0
    p2t = np.zeros((128, 128), dtype=np.float32)
    p2t[0:D, 0:D] = P64
    p2t[D:128, D:128] = P64
    # diag-band masks: msk[p][key, q] = 1 iff q - key >= 128*p
    k_idx = np.arange(KT)[:, None]
    q_idx = np.arange(QB)[None, :]
    msk = np.stack(
        [(q_idx - k_idx >= 128 * p).astype(np.float32) for p in range(4)],
        axis=1,
    ).reshape(KT, 4 * QB)
    return cos128, gsin128, p2t.astype(BF), msk.astype(BF)


def _pack_w(w_qkv, heads):
    """Pack this core's q/k rows into the (384, C) tile layout and v rows
    into (192, C)."""
    q = [w_qkv[0 * C + h * D: 0 * C + (h + 1) * D] for h in heads]
    kk = [w_qkv[1 * C + h * D: 1 * C + (h + 1) * D] for h in heads]
    v = [w_qkv[2 * C + h * D: 2 * C + (h + 1) * D] for h in heads]
    wqk = np.concatenate([q[0], q[1], kk[0], kk[1], q[2], kk[2]], axis=0)
    wv = np.concatenate(v, axis=0)
    return wqk, wv


def _make_in_maps(x, w_qkv, w_proj, t_len=T):
    cos128, gsin128, p2t, msk = _host_consts(t_len)
    in_maps = []
    for core in range(NCORES):
        b, hg = divmod(core, 4)
        heads = list(range(hg * HPC, (hg + 1) * HPC))
        wqk, wv = _pack_w(w_qkv, heads)
        cs = slice(hg * HPC * D, (hg + 1) * HPC * D)
        in_maps.append(
            {
                "xT": np.ascontiguousarray(x[b].T).astype(BF),
                "wqkT": np.ascontiguousarray(wqk.T).astype(BF),
                "wvT": np.ascontiguousarray(wv.T).astype(BF),
                "wpT": np.ascontiguousarray(w_proj[:, cs].T).astype(BF),
                "cosT": cos128, "gsinT": gsin128, "p2t": p2t, "msk": msk,
            }
        )
    return in_maps


def kernel(x, w_qkv, w_proj):
    x = np.asarray(x, dtype=np.float32)
    w_qkv = np.asarray(w_qkv, dtype=np.float32)
    w_proj = np.asarray(w_proj, dtype=np.float32)

    in_maps = _make_in_maps(x, w_qkv, w_proj)
    nc = _get_nc()
    res = run_bass_kernel_spmd(nc, in_maps, core_ids=list(range(NCORES)))
    out = np.zeros((B, T, C), dtype=np.float32)
    for core in range(NCORES):
        b = core // 4
        out[b] += res.results[core]["outT"].T.astype(np.float32)
    return out


# revision 37
# speedup vs baseline: 1.0471x; 1.0053x over previous
"""Causal self-attention with RoPE, sharded over 8 TRN2 NeuronCores.

Sharding: core = (batch b, head-group hg). Cores 0-3 -> batch 0, cores 4-7 ->
batch 1; head-group hg = core % 4 owns heads [3*hg, 3*hg+3). Each core computes
its heads' attention and a partial output projection (w_proj column-slice);
the host sums the 4 partials per batch (the row-sharded projection's
all-reduce, done on host since full outputs are gathered anyway).

Per-core kernel, bf16 data / fp32 PSUM accumulation. Matmul outputs are
hard-capped at one PSUM bank (512 fp32/partition), so all matmuls run
512-wide; non-matmul ops (muls, adds, copies, masks, exp, divides) span
two banks per instruction to halve instruction count:
  - QKV q/k features packed in 3x128 tiles [q0|q1] [k0|k1] [q2|k2],
    processed per token-block PAIR: two 512-wide matmul groups accumulate
    into one 2-bank tile, then single 1024-wide RoPE ops run on it.
  - RoPE with signs folded into a permuted sin table: q' = (acc*cos) +
    P2T @ (acc*gsin), P2T the plain half-swap permutation, gsin =
    [+sin; -sin] per 32-half. The rotate matmuls lag one acc-group so
    the PE never waits on the DVE muls. k2 is mirrored into a 4th qkrot
    slot's low half (SBUF->SBUF DMA) for head-2's equal-base-partition
    scores.
  - V^T computed directly: vt[t,d] = sum_c xT[c,t]*wv[c,d]; two key
    tiles' results share one PSUM bank and one copy into the augmented-V
    buffer whose 65th column of ones yields the softmax denominator.
  - Attention in scores-transposed layout [keys, queries]: probsT =
    exp(K^T.T @ Q^T * 0.125) on ACT over 2-tile groups, causal
    diagonal-band masking one instruction per group (2D-affine
    gpsimd affine_select alternating with DVE multiply by precomputed
    0/1 masks). PV accumulates over key tiles, software-pipelined with
    a 2-group lookahead behind scores.
  - The divide by the softmax denominator is folded into the PSUM->SBUF
    move.
  - Projection: partial outT = wpT.T @ attnT into a persistent buffer
    (paired 2-bank tiles, 1024-wide copies), 6 large bf16 DMAs out;
    host sums the 4 partials per batch in fp32.

All SBUF tile pools open once, outside the For_i timing loop (no pool
boundaries inside an iteration); only the per-phase PSUM pools (bank
reuse: 8/8/6 banks) open inside.
"""

import numpy as np
import ml_dtypes

import concourse.bass as bass
import concourse.bacc as bacc
import concourse.tile as tile
from concourse import mybir
from concourse.bass_utils import run_bass_kernel_spmd

B, T, C, H = 2, 2048, 768, 12
D = C // H  # 64
ROPE_THETA = 10000.0
NCORES = 8
HPC = 3             # heads per core
NQF = 3             # packed q/k feature tiles: [q0|q1] [k0|k1] [q2|k2]
QB = 512            # query block (free dim of scores^T tiles)
KT = 128            # key tile (partition dim of scores^T tiles)

F32 = mybir.dt.float32
BF16 = mybir.dt.bfloat16
BF = ml_dtypes.bfloat16

# (feature-tile, half) of each head's q / k block in the packed layout
Q_POS = {0: (0, 0), 1: (0, 1), 2: (2, 0)}
K_POS = {0: (1, 0), 1: (1, 1), 2: (3, 0)}   # k2 DMA-copied to slot 3 lo-half


def _build_nc(t_len=T, loops=1, unroll=False, body=1):
    nc = bacc.Bacc("TRN2", target_bir_lowering=False, debug=False)

    xT_d = nc.dram_tensor("xT", [C, t_len], BF16, kind="ExternalInput")
    wqk_d = nc.dram_tensor("wqkT", [C, NQF * 128], BF16, kind="ExternalInput")
    wv_d = nc.dram_tensor("wvT", [C, HPC * D], BF16, kind="ExternalInput")
    wp_d = nc.dram_tensor("wpT", [HPC * D, C], BF16, kind="ExternalInput")
    cos_d = nc.dram_tensor("cosT", [128, t_len], F32, kind="ExternalInput")
    gsin_d = nc.dram_tensor("gsinT", [128, t_len], F32, kind="ExternalInput")
    p2t_d = nc.dram_tensor("p2t", [128, 128], BF16, kind="ExternalInput")
    msk_d = nc.dram_tensor("msk", [128, 4 * QB], BF16, kind="ExternalInput")
    outT_d = nc.dram_tensor("outT", [C, t_len], BF16, kind="ExternalOutput")

    with tile.TileContext(nc) as tc:
        _body(tc, t_len, xT_d, wqk_d, wv_d, wp_d, cos_d, gsin_d, p2t_d, msk_d,
              outT_d, loops=loops, unroll=unroll, body=body)
    nc.compile()
    return nc


def _body(tc, t_len, xT_d, wqk_d, wv_d, wp_d, cos_d, gsin_d, p2t_d, msk_d,
          outT_d, loops=1, unroll=False, body=1):
    T = t_len
    NCT = C // 128
    NKT = T // KT
    with (
        tc.tile_pool(name="singles", bufs=1) as singles,
        tc.tile_pool(name="sb_x", bufs=2) as sb_x,
        tc.tile_pool(name="sb_qs", bufs=3) as sb_qs,
        tc.tile_pool(name="sb_qc", bufs=3) as sb_qc,
        tc.tile_pool(name="sb_probs", bufs=6) as sb_probs,
        tc.tile_pool(name="sb_rcp", bufs=2) as sb_rcp,
    ):
        s = {}
        s["wqk"] = singles.tile([128, NCT, NQF * 128], BF16, tag="wqk", name="wqk")
        s["wv"] = singles.tile([128, NCT, HPC * D], BF16, tag="wv", name="wv")
        s["wp0"] = singles.tile([128, C], BF16, tag="wp0", name="wp0")
        s["wp1"] = singles.tile([64, C], BF16, tag="wp1", name="wp1")
        s["cosc"] = singles.tile([128, T], F32, tag="cosc", name="cosc")
        s["gsin"] = singles.tile([128, T], F32, tag="gsin", name="gsin")
        s["p2t"] = singles.tile([128, 128], BF16, tag="p2t", name="p2t")
        s["msk"] = singles.tile([128, 4, QB], BF16, tag="msk", name="msk")
        s["qkrot"] = singles.tile([128, NQF + 1, T], BF16, tag="qkrot", name="qkrot")
        s["va"] = singles.tile([128, NKT, HPC, D + 1], BF16, tag="va", name="va")
        s["at01"] = singles.tile([128, T], BF16, tag="at01", name="at01")
        s["at2"] = singles.tile([64, T], BF16, tag="at2", name="at2")
        s["outb"] = singles.tile([128, C // 128, T], BF16, tag="outb", name="outb")
        pools = dict(sb_x=sb_x, sb_qs=sb_qs, sb_qc=sb_qc, sb_probs=sb_probs,
                     sb_rcp=sb_rcp)

        # pre-zero the probs ring once so trimmed-range exp never leaves
        # non-finite garbage in masked regions (mask-multiply safe)
        for i in range(6):
            pz = sb_probs.tile([128, 2, QB], BF16, tag="probs", name=f"pz{i}")
            nc = tc.nc
            nc.vector.memset(pz, 0.0)

        if loops > 1 and unroll:
            for _ in range(loops * body):
                _compute(tc, t_len, s, pools, xT_d, wqk_d, wv_d, wp_d, cos_d,
                         gsin_d, p2t_d, msk_d, outT_d)
        elif loops > 1:
            with tc.For_i(0, loops, 1):
                for _ in range(body):
                    _compute(tc, t_len, s, pools, xT_d, wqk_d, wv_d, wp_d,
                             cos_d, gsin_d, p2t_d, msk_d, outT_d)
        else:
            for _ in range(body):
                _compute(tc, t_len, s, pools, xT_d, wqk_d, wv_d, wp_d, cos_d,
                         gsin_d, p2t_d, msk_d, outT_d)


def _compute(tc, t_len, s, pools, xT_d, wqk_d, wv_d, wp_d, cos_d, gsin_d,
             p2t_d, msk_d, outT_d):
    nc = tc.nc
    T = t_len
    NQB = T // QB       # 4 query blocks
    NKT = T // KT       # 16 key tiles
    NCT = C // 128      # 6 contraction tiles over channels
    JPB = QB // KT      # key tiles per query block (4)
    QB2 = 2 * QB        # elementwise ops span two 512 tiles
    NTB2 = T // QB2     # 2 token-block pairs

    wqk, wv, wp0, wp1 = s["wqk"], s["wv"], s["wp0"], s["wp1"]
    cosc, gsin, p2t, msk = s["cosc"], s["gsin"], s["p2t"], s["msk"]
    qkrot, va, at01, at2, outb = (s["qkrot"], s["va"], s["at01"], s["at2"],
                                  s["outb"])
    sb_x, sb_qs, sb_qc = pools["sb_x"], pools["sb_qs"], pools["sb_qc"]
    sb_probs, sb_rcp = pools["sb_probs"], pools["sb_rcp"]

    wqk_v = wqk_d.ap().rearrange("(a p) f -> p a f", p=128)
    wv_v = wv_d.ap().rearrange("(a p) f -> p a f", p=128)
    xT_v = xT_d.ap().rearrange("(a p) t -> p a t", p=128)

    xtbs = []
    for tb in range(NQB):
        xtb = sb_x.tile([128, NCT, QB], BF16, tag="xtb", name=f"xtb{tb}")
        xtbs.append(xtb)
    # critical-path order: weights + x block 0 + rope tables first, the
    # remaining x blocks next, cold constants trailing
    nc.sync.dma_start(out=wqk, in_=wqk_v)
    nc.sync.dma_start(out=xtbs[0], in_=xT_v[:, :, 0:QB])
    nc.sync.dma_start(out=cosc, in_=cos_d.ap())
    nc.sync.dma_start(out=gsin, in_=gsin_d.ap())
    nc.sync.dma_start(out=p2t, in_=p2t_d.ap())
    nc.sync.dma_start(out=wv, in_=wv_v)
    nc.sync.dma_start(out=xtbs[1], in_=xT_v[:, :, QB:2 * QB])
    nc.sync.dma_start(out=xtbs[2], in_=xT_v[:, :, 2 * QB:3 * QB])
    nc.sync.dma_start(out=xtbs[3], in_=xT_v[:, :, 3 * QB:4 * QB])
    nc.sync.dma_start(out=msk, in_=msk_d.ap().rearrange("p (a q) -> p a q", q=QB))
    nc.sync.dma_start(out=wp0, in_=wp_d.ap()[0:128, :])
    nc.sync.dma_start(out=wp1, in_=wp_d.ap()[128:192, :])

    # ones column of the augmented V tiles (softmax denominator)
    nc.vector.memset(va[:, :, :, D], 1.0)

    # ---- QKV projection + RoPE + direct V^T ------------------------------
    with (
        tc.tile_pool(name="ps_a", bufs=2, space="PSUM") as ps_a,
        tc.tile_pool(name="ps_r", bufs=2, space="PSUM") as ps_r,
        tc.tile_pool(name="ps_v", bufs=3, space="PSUM") as ps_v,
    ):
        # software pipeline: the rotate matmul for (tb, ft) is emitted one
        # acc-group later so the PE never waits on the DVE mul feeding it
        prev = []   # at most one (qs, qc, ft, tb) awaiting rotate

        def flush_rot():
            qs, qc, ft, tb = prev.pop(0)
            ts = slice(tb * QB, (tb + 1) * QB)
            rh = ps_r.tile([128, QB], F32, tag="rh", name="rh")
            nc.tensor.matmul(rh, p2t, qs, start=True, stop=True)
            nc.vector.tensor_add(qkrot[:, ft, ts], qc, rh)
            if ft == 2:
                # scores need k2 on the same base partition as q2:
                # mirror the hi half into slot 3's lo half
                nc.sync.dma_start(out=qkrot[0:64, 3, ts],
                                  in_=qkrot[64:128, 2, ts])

        for tb in range(NQB):
            ts = slice(tb * QB, (tb + 1) * QB)
            xtb = xtbs[tb]
            for ft in range(NQF):
                acc = ps_a.tile([128, QB], F32, tag="acc", name="acc")
                for ct in range(NCT):
                    nc.tensor.matmul(
                        acc,
                        wqk[:, ct, ft * 128: (ft + 1) * 128],
                        xtb[:, ct, :],
                        start=(ct == 0),
                        stop=(ct == NCT - 1),
                    )
                if prev:
                    flush_rot()
                qs = sb_qs.tile([128, QB], BF16, tag="qs", name="qs")
                nc.vector.tensor_mul(qs, acc, gsin[:, ts])
                qc = sb_qc.tile([128, QB], BF16, tag="qc", name="qc")
                nc.vector.tensor_mul(qc, acc, cosc[:, ts])
                prev.append((qs, qc, ft, tb))
            # direct V^T for this token block (4 x 128-token tiles, two
            # per PSUM bank / per copy)
            for j2 in range(JPB // 2):
                kt = tb * JPB + 2 * j2
                vt = ps_v.tile([128, 2, HPC * D], F32, tag="vt", name="vt")
                for u in range(2):
                    for ct in range(NCT):
                        nc.tensor.matmul(
                            vt[:, u, :],
                            xtb[:, ct, (2 * j2 + u) * KT: (2 * j2 + u + 1) * KT],
                            wv[:, ct, :],
                            start=(ct == 0),
                            stop=(ct == NCT - 1),
                        )
                if j2 == 0 and prev:
                    flush_rot()
                nc.scalar.copy(
                    va[:, kt: kt + 2, :, 0:D],
                    vt.rearrange("p a (h d) -> p a h d", h=HPC),
                )
        while prev:
            flush_rot()

    # ---- attention -------------------------------------------------------
    def qk_ap(pos, ts):
        ti, half = pos
        return qkrot[half * 64: half * 64 + 64, ti, ts]

    with (
        tc.tile_pool(name="ps_sc", bufs=3, space="PSUM") as ps_sc,
        tc.tile_pool(name="ps_pv", bufs=2, space="PSUM") as ps_pv,
    ):
        pvs = {}     # (h, qb) -> pv tile
        pend = []    # (h, qb, g, probs2, nkt) score groups awaiting PV
        nmask = [0]  # running diag-mask count for engine alternation

        def emit_pv(depth):
            while len(pend) > depth:
                h, qb, g, probs2, nkt = pend.pop(0)
                pv = pvs[(h, qb)]
                for j2 in range(2):
                    kt = g * 2 + j2
                    nc.tensor.matmul(
                        pv,
                        va[:, kt, h, :],
                        probs2[:, j2, :],
                        start=(kt == 0),
                        stop=(kt == nkt - 1),
                    )
                if g == nkt // 2 - 1:
                    # block done: fold denominator into the PSUM->SBUF move
                    rcp = sb_rcp.tile([1, QB], F32, tag="rcp", name="rcp")
                    nc.vector.reciprocal(rcp, pv[64:65, :])
                    rcpb = sb_rcp.tile([64, QB], F32, tag="rcpb", name="rcpb")
                    nc.gpsimd.partition_broadcast(rcpb, rcp)
                    qs_ = slice(qb * QB, (qb + 1) * QB)
                    if h == 0:
                        dst = at01[0:64, qs_]
                    elif h == 1:
                        dst = at01[64:128, qs_]
                    else:
                        dst = at2[:, qs_]
                    nc.vector.tensor_mul(dst, pv[0:64, :], rcpb)
                    del pvs[(h, qb)]

        for h in range(HPC):
            for qb in range(NQB):
                qs_ = slice(qb * QB, (qb + 1) * QB)
                nkt = (qb + 1) * JPB
                pvs[(h, qb)] = ps_pv.tile([65, QB], F32, tag="pv",
                                          name=f"pv{h}_{qb}")
                for g in range(nkt // 2):
                    sc2 = ps_sc.tile([128, 2, QB], F32, tag="sc2", name="sc2")
                    for j2 in range(2):
                        kt = g * 2 + j2
                        nc.tensor.matmul(
                            sc2[:, j2, :],
                            qk_ap(K_POS[h], slice(kt * KT, (kt + 1) * KT)),
                            qk_ap(Q_POS[h], qs_),
                            start=True, stop=True,
                        )
                    probs2 = sb_probs.tile([128, 2, QB], BF16, tag="probs",
                                           name="probs")
                    p0 = g * 2 - qb * JPB  # diag-band pattern of tile j2=0
                    # diag-band groups: columns < 128*p0 are masked in BOTH
                    # tiles, so exp only [128*p0, 512) of each; the
                    # affine_select zero-fill covers the unwritten range
                    lo = max(p0, 0) * KT
                    nc.scalar.activation(
                        probs2[:, :, lo:QB], sc2[:, :, lo:QB],
                        mybir.ActivationFunctionType.Exp,
                        scale=float(1.0 / np.sqrt(D)),
                    )
                    for j2 in range(2):
                        kt = g * 2 + j2
                        p = kt - qb * JPB  # diag-band pattern id
                        if p >= 0:
                            nc.vector.tensor_mul(
                                probs2[:, j2, :],
                                probs2[:, j2, :],
                                msk[:, p, :],
                            )
                    pend.append((h, qb, g, probs2, nkt))
                    emit_pv(2)
        emit_pv(0)

    # ---- output projection (partial over this core's 192 channels) -------
    with tc.tile_pool(name="ps_po", bufs=4, space="PSUM") as ps_po:
        k = 0
        for co in range(C // 128):
            for tb in range(NQB):
                ts = slice(tb * QB, (tb + 1) * QB)
                po = ps_po.tile([128, QB], F32, tag="po", name="po")
                nc.tensor.matmul(
                    po, wp0[:, co * 128: (co + 1) * 128],
                    at01[:, ts], start=True, stop=False,
                )
                nc.tensor.matmul(
                    po, wp1[:, co * 128: (co + 1) * 128],
                    at2[:, ts], start=False, stop=True,
                )
                ot = outb[:, co, ts]
                if k % 2 == 0:
                    nc.vector.tensor_copy(ot, po)
                else:
                    nc.scalar.copy(ot, po)
                k += 1
            nc.scalar.dma_start(
                out=outT_d.ap()[co * 128: (co + 1) * 128, :],
                in_=outb[:, co, :],
            )


_NC_CACHE = {}


def _get_nc():
    if "nc" not in _NC_CACHE:
        _NC_CACHE["nc"] = _build_nc()
    return _NC_CACHE["nc"]


def _host_consts(t_len=T):
    half = D // 2  # 32
    inv_freq = 1.0 / (ROPE_THETA ** (np.arange(0, D, 2, dtype=np.float32) / D))
    ang = np.arange(t_len, dtype=np.float32)[:, None] * inv_freq[None, :]
    sin = np.sin(ang).T.astype(np.float32)   # (32, T)
    cos = np.cos(ang).T.astype(np.float32)   # (32, T)
    cos64 = np.concatenate([cos, cos], axis=0)            # (64, T)
    gsin64 = np.concatenate([sin, -sin], axis=0)          # (64, T)
    cos128 = np.concatenate([cos64, cos64], axis=0)       # (128, T)
    gsin128 = np.concatenate([gsin64, gsin64], axis=0)    # (128, T)
    # plain half-swap permutation per 64-feature block:
    # out[m] = in[sigma(m)], sigma swaps 32-halves; p2t[sigma(m), m] = 1
    P64 = np.zeros((D, D), dtype=np.float32)
    P64[np.arange(half), np.arange(half) + half] = 1.0
    P64[np.arange(half) + half, np.arange(half)] = 1.0
    p2t = np.zeros((128, 128), dtype=np.float32)
    p2t[0:D, 0:D] = P64
    p2t[D:128, D:128] = P64
    # diag-band masks: msk[p][key, q] = 1 iff q - key >= 128*p
    k_idx = np.arange(KT)[:, None]
    q_idx = np.arange(QB)[None, :]
    msk = np.stack(
        [(q_idx - k_idx >= 128 * p).astype(np.float32) for p in range(4)],
        axis=1,
    ).reshape(KT, 4 * QB)
    return cos128, gsin128, p2t.astype(BF), msk.astype(BF)


def _pack_w(w_qkv, heads):
    """Pack this core's q/k rows into the (384, C) tile layout and v rows
    into (192, C)."""
    q = [w_qkv[0 * C + h * D: 0 * C + (h + 1) * D] for h in heads]
    kk = [w_qkv[1 * C + h * D: 1 * C + (h + 1) * D] for h in heads]
    v = [w_qkv[2 * C + h * D: 2 * C + (h + 1) * D] for h in heads]
    wqk = np.concatenate([q[0], q[1], kk[0], kk[1], q[2], kk[2]], axis=0)
    wv = np.concatenate(v, axis=0)
    return wqk, wv


def _make_in_maps(x, w_qkv, w_proj, t_len=T):
    cos128, gsin128, p2t, msk = _host_consts(t_len)
    in_maps = []
    for core in range(NCORES):
        b, hg = divmod(core, 4)
        heads = list(range(hg * HPC, (hg + 1) * HPC))
        wqk, wv = _pack_w(w_qkv, heads)
        cs = slice(hg * HPC * D, (hg + 1) * HPC * D)
        in_maps.append(
            {
                "xT": np.ascontiguousarray(x[b].T).astype(BF),
                "wqkT": np.ascontiguousarray(wqk.T).astype(BF),
                "wvT": np.ascontiguousarray(wv.T).astype(BF),
                "wpT": np.ascontiguousarray(w_proj[:, cs].T).astype(BF),
                "cosT": cos128, "gsinT": gsin128, "p2t": p2t, "msk": msk,
            }
        )
    return in_maps


def kernel(x, w_qkv, w_proj):
    x = np.asarray(x, dtype=np.float32)
    w_qkv = np.asarray(w_qkv, dtype=np.float32)
    w_proj = np.asarray(w_proj, dtype=np.float32)

    in_maps = _make_in_maps(x, w_qkv, w_proj)
    nc = _get_nc()
    res = run_bass_kernel_spmd(nc, in_maps, core_ids=list(range(NCORES)))
    out = np.zeros((B, T, C), dtype=np.float32)
    for core in range(NCORES):
        b = core // 4
        out[b] += res.results[core]["outT"].T.astype(np.float32)
    return out
